# revision 1
# baseline (speedup 1.0000x reference)
"""MiniTransformer block on 8 Trainium2 NeuronCores.

Sharding: pure data-parallel over batch (B=8 -> 1 batch element per core,
no collectives). Per core the full transformer block (LN -> single-head
attention -> residual -> LN -> MLP -> residual) runs as one Bass/Tile kernel.

Key design points:
  * All matmuls run in float32r (TF32-like, 1 cycle/row on the PE at free
    dim >= 256 vs 4 cycles/row for fp32; measured fro rel err ~1.5e-4).
  * Activations for matmul consumption are kept transposed ([feature, token])
    so projections chain without transposes; only LN outputs are transposed
    (PE transpose, 4 per 128-row chunk).
  * Attention scores are computed via a host-folded Wu = Wk @ Wq^T:
    scores^T = (h Wu) . h, so only ONE projection (u) is materialized
    instead of q and k.
  * Softmax: scores are computed transposed [t, s]; exp (with the 1/sqrt(D)
    scale fused) happens on the ScalarE during PSUM eviction; no max
    subtraction (LN-bounded scores, fp32 exp range is ample); the
    denominator comes from an extra ones-column appended to v, landing in
    PSUM as a per-partition scalar; normalization + residual add fold into
    a single scalar_tensor_tensor eviction.
  * (p @ v) @ Wo is computed as p @ (v (Wv Wo)) via host-folded Wvo,
    removing a projection and a transpose.
  * The attention side (u/v projections, scores, p@v) runs in fp8e4m3 with
    DoubleRow matmuls (2 contraction rows/cycle, 2x the f32r rate). LN keeps
    activations in fp8 range; exp is biased by ln(1/16) so p~ = exp(s/23)/16
    stays < 240 (fp8 max) -- softmax normalization cancels the constant.
    Attention here is diffuse (scores O(1)), so fp8 score noise averages out.
    The MLP runs with bf16 weights/activations (W1/W2/h2T/gT) but f32 PSUM
    accumulation -- fp8 there would cost ~3e-2 rel err (quantization noise of
    a random linear map does not average down), bf16 only ~5e-4. Measured
    total rel err 3.9e-3 (vs 1.2e-4 all-f32r), budget 2e-2.
  * DMA traffic is spread across the SP/ACT HWDGE queues and the Pool SWDGE
    queue so no engine's sequencer stalls compute.

Host/dispatch side (the measured wall time is dominated by the axon tunnel's
~80 ms round-trip latency plus per-call dispatch cost, not kernel time):
  * Inputs are packed into 3 operands (x, fp8 wpack8, f32 wpack + cpack) so
    the per-call operand marshaling is minimal.
  * The runner is AOT-compiled via fast_dispatch_compile, which suppresses
    the bass_exec jax effect and enables C++ fast-path dispatch (~0.2 ms/call
    vs ~1.2 ms on the effectful python path).
"""

import numpy as np

S, D, F, P = 2048, 512, 2048, 128
SC, DC, FC = S // P, D // P, F // P  # 16, 4, 16
SB = 512                             # attention s-block
NB = S // SB                         # 4
CPB = SB // P                        # s-chunks per attention block = 4
SBM = 512                            # MLP s-block
NBM = S // SBM                       # 4
CPBM = SBM // P                      # s-chunks per MLP block = 4
NCORES = 8
LN_EPS = 1e-5
ATTN_SCALE = float(1.0 / np.sqrt(np.float32(D)))
LOG_EXP_C = float(np.log(1.0 / 16.0))

_CACHE = {}


def _build(has_affine1, has_affine2):
    import concourse.bass as bass
    import concourse.mybir as mybir
    import concourse.tile as tile
    from concourse import bacc
    from concourse.masks import make_identity
    from contextlib import ExitStack

    f32 = mybir.dt.float32
    f32r = mybir.dt.float32r
    f8 = mybir.dt.float8e4
    bf16 = mybir.dt.bfloat16
    PM2 = mybir.MatmulPerfMode.DoubleRow
    AF = mybir.ActivationFunctionType
    OP = mybir.AluOpType

    nc = bacc.Bacc("TRN2", target_bir_lowering=False, debug=False,
                   num_devices=NCORES)

    # Inputs are packed into three flat buffers (x aside) so the per-call
    # dispatch streams 5 operand handles instead of 13: "wpack8" carries the
    # fp8 attention weights, "wpack" the bf16 MLP weights, "cpack" the six
    # small f32 vectors. Views with the original access patterns are
    # hand-built APs at element offsets.
    x_d = nc.dram_tensor("x", [S, D], f32, kind="ExternalInput").ap()
    wpack8_d = nc.dram_tensor("wpack8", [2 * D * D], f8,
                              kind="ExternalInput").ap()
    wpack_d = nc.dram_tensor("wpack", [2 * D * F], bf16,
                             kind="ExternalInput").ap()
    cpack_d = nc.dram_tensor("cpack", [F + 5 * D], f32,
                             kind="ExternalInput").ap()
    out_d = nc.dram_tensor("out", [S, D], f32, kind="ExternalOutput").ap()

    def w8view(base, dims):
        return bass.AP(tensor=wpack8_d.tensor, offset=base,
                       ap=[[s, n] for s, n in dims])

    def wview(base, dims):
        return bass.AP(tensor=wpack_d.tensor, offset=base,
                       ap=[[s, n] for s, n in dims])

    def cview(base, dims):
        return bass.AP(tensor=cpack_d.tensor, offset=base,
                       ap=[[s, n] for s, n in dims])

    x_r = x_d.rearrange("(sc p) d -> p sc d", p=P)      # [128, 16, 512]
    out_r = out_d.rearrange("(sc p) d -> p sc d", p=P)
    # layouts match the originals: wu/wvo [D,D] "(ko ki) n -> ki ko n",
    # w1 [D,F] likewise, w2 [F,D] likewise (ki = partition dim = 128)
    wu_r = w8view(0, [(D, P), (P * D, DC), (1, D)])         # [128, 4, 512]
    wvo_r = w8view(D * D, [(D, P), (P * D, DC), (1, D)])
    w1_r = wview(0, [(F, P), (P * F, DC), (1, F)])          # [128, 4, 2048]
    w2_r = wview(D * F, [(D, P), (P * D, FC), (1, D)])
    bf_r = cview(0, [(1, P), (P, FC)])                      # [128, 16]
    b2_d = cview(F, [(0, P), (1, D)])                       # bcast views
    g1_d = cview(F + D, [(0, P), (1, D)])
    be1_d = cview(F + 2 * D, [(0, P), (1, D)])
    g2_d = cview(F + 3 * D, [(0, P), (1, D)])
    be2_d = cview(F + 4 * D, [(0, P), (1, D)])

    def bcast(ap):  # packed views above are already partition-broadcast
        return ap

    with tile.TileContext(nc) as tc, ExitStack() as top:
        long_pool = top.enter_context(tc.tile_pool(name="long", bufs=1))
        const_pool = top.enter_context(tc.tile_pool(name="consts", bufs=1))
        w1_pool = top.enter_context(tc.tile_pool(name="w1p", bufs=1))
        tmpBC = top.enter_context(tc.tile_pool(name="tmpBC", bufs=2))

        # ---- constants / small tiles -------------------------------------
        ident = const_pool.tile([P, P], f32r)
        with tc.tile_pool(name="identf", bufs=1) as idp:
            ident_f = idp.tile([P, P], f32)
            make_identity(nc, ident_f[:])
            nc.vector.tensor_copy(ident[:], ident_f[:])
        # PE warmup: dependency-free matmuls at t=0 release the HAM throttle
        # (~3.4us of sustained PE activity) before real work arrives.
        with tc.tile_pool(name="warmps", bufs=1, space="PSUM") as wps:
            wm = wps.tile([P, 512], f32)
            for _ in range(16):
                nc.tensor.matmul(wm[:, 0:P], ident[:], ident[:],
                                 start=True, stop=True)
        eps_t = const_pool.tile([P, 1], f32)
        nc.vector.memset(eps_t[:], LN_EPS)
        # exp bias ln(1/16): p~ = exp(s/sqrt(D))/16 stays < 240 (fp8e4 max);
        # softmax normalization cancels the constant exactly.
        lnc_t = const_pool.tile([P, 1], f32)
        nc.vector.memset(lnc_t[:], LOG_EXP_C)
        ones2 = const_pool.tile([P, 2], f32)
        nc.vector.memset(ones2[:, 0:1], 1.0)
        nc.vector.memset(ones2[:, 1:2], 0.0)
        bf_t = const_pool.tile([P, FC], f32)
        nc.gpsimd.dma_start(bf_t[:], bf_r)
        b2_t = const_pool.tile([P, D], f32)
        nc.gpsimd.dma_start(b2_t[:], bcast(b2_d))
        aff = {}
        if has_affine1:
            aff[1] = (const_pool.tile([P, D], f32, tag="g1b"),
                      const_pool.tile([P, D], f32, tag="be1b"))
            nc.gpsimd.dma_start(aff[1][0][:], bcast(g1_d))
            nc.gpsimd.dma_start(aff[1][1][:], bcast(be1_d))
        if has_affine2:
            aff[2] = (const_pool.tile([P, D], f32, tag="g2b"),
                      const_pool.tile([P, D], f32, tag="be2b"))
            nc.gpsimd.dma_start(aff[2][0][:], bcast(g2_d))
            nc.gpsimd.dma_start(aff[2][1][:], bcast(be2_d))

        # LN2 batched stats (written in phase B, consumed in phase C)
        mv_all = const_pool.tile([P, SC, 2], f32)
        rstd_all = const_pool.tile([P, SC], f32)
        std_all = const_pool.tile([P, SC], f32)

        # ---- persistent activations / preloaded weights ------------------
        xb = long_pool.tile([P, SC, D], f32)             # x, becomes x2 in place
        hT = long_pool.tile([P, DC, S], f8, tag="actT")  # LN1 output, fp8
        w1_t = w1_pool.tile([P, DC, F], bf16)

        def ln_apply(tmp_pool, i, mean, rstd, which):
            """(xb[:,i,:] - mean) * rstd [*gamma + beta] -> row-major tile."""
            h_t = tmp_pool.tile([P, D], f32r, tag="h_rm")
            nc.vector.tensor_scalar(out=h_t[:], in0=xb[:, i, :],
                                    scalar1=mean, scalar2=rstd,
                                    op0=OP.subtract, op1=OP.mult)
            if which in aff:
                g_b, be_b = aff[which]
                nc.vector.tensor_tensor(h_t[:], h_t[:], g_b[:], op=OP.mult)
                nc.vector.tensor_tensor(h_t[:], h_t[:], be_b[:], op=OP.add)
            return h_t

        def transpose_to(tr_psum, dest_T, i, h_t):
            for dj in range(DC):
                ps = tr_psum.tile([P, P], f32r, tag="tr")
                nc.tensor.transpose(ps[:], h_t[:, dj * P:(dj + 1) * P], ident[:])
                nc.vector.tensor_copy(dest_T[:, dj, i * P:(i + 1) * P], ps[:])

        # ================= phase A: LN1 + u/v projections =================
        with ExitStack() as ph:
            ph_qk = ph.enter_context(tc.tile_pool(name="uv", bufs=1))
            uT = ph_qk.tile([P, DC, S], f8, tag="uT")
            v_aug = ph_qk.tile([P, SC, D + 2], f8, tag="vaug")

            with ExitStack() as pha:
                wA_pool = pha.enter_context(tc.tile_pool(name="wA", bufs=1))
                tmpA = pha.enter_context(tc.tile_pool(name="tmpA", bufs=3))
                statsA = pha.enter_context(tc.tile_pool(name="statsA", bufs=4))
                tr_psA = pha.enter_context(tc.tile_pool(name="trpsA", bufs=2,
                                                        space="PSUM"))
                mm_psA = pha.enter_context(tc.tile_pool(name="mmpsA", bufs=5,
                                                        space="PSUM"))

                wu_t = wA_pool.tile([P, DC, D], f8)
                nc.gpsimd.dma_start(wu_t[:], wu_r)
                wvo_t = wA_pool.tile([P, DC, D], f8)
                nc.gpsimd.dma_start(wvo_t[:], wvo_r)
                # W1 preload early on the idle-after-that Pool queue
                nc.gpsimd.dma_start(w1_t[:], w1_r)

                for i in range(SC):
                    dma_eng = nc.sync if i % 2 == 0 else nc.scalar
                    dma_eng.dma_start(xb[:, i, :], x_r[:, i, :])
                    stats = statsA.tile([P, 6], f32, tag="bn_stats")
                    nc.vector.bn_stats(stats[:], xb[:, i, :])
                    mv = statsA.tile([P, 2], f32, tag="bn_aggr")
                    nc.vector.bn_aggr(mv[:], stats[:])
                    std = statsA.tile([P, 1], f32, tag="std")
                    nc.scalar.activation(std[:], mv[:, 1:2], AF.Sqrt,
                                         bias=eps_t[:], scale=1.0)
                    rstd = statsA.tile([P, 1], f32, tag="rstd")
                    nc.vector.reciprocal(rstd[:], std[:])
                    h_t = ln_apply(tmpA, i, mv[:, 0:1], rstd[:], 1)
                    transpose_to(tr_psA, hT, i, h_t)
                    # v' row-major for this t-chunk: [t, dout] = h @ Wvo
                    # (fp8 DoubleRow: two k-chunks contracted per matmul)
                    ps = mm_psA.tile([P, 512], f32, tag="proj")
                    for k in range(DC // 2):
                        nc.tensor.matmul(ps[:],
                                         hT[:, 2 * k:2 * k + 2,
                                            i * P:(i + 1) * P],
                                         wvo_t[:, 2 * k:2 * k + 2, :],
                                         start=(k == 0),
                                         stop=(k == DC // 2 - 1),
                                         perf_mode=PM2)
                    nc.vector.tensor_copy(v_aug[:, i, 0:D], ps[:])
                    # uT s-tile as soon as its 4 h-chunks exist
                    if i % 4 == 3:
                        n = i // 4
                        for m in range(DC):
                            ps = mm_psA.tile([P, 512], f32, tag="proj")
                            for k in range(DC // 2):
                                nc.tensor.matmul(
                                    ps[:],
                                    wu_t[:, 2 * k:2 * k + 2,
                                         m * P:(m + 1) * P],
                                    hT[:, 2 * k:2 * k + 2,
                                       n * 512:(n + 1) * 512],
                                    start=(k == 0), stop=(k == DC // 2 - 1),
                                    perf_mode=PM2)
                            nc.vector.tensor_copy(
                                uT[:, m, n * 512:(n + 1) * 512], ps[:])
                # ones/zero columns for the softmax denominator
                nc.vector.tensor_copy(
                    v_aug[:, :, D:D + 2],
                    ones2[:, None, :].to_broadcast((P, SC, 2)))

            # ============= phase B: attention (+ LN2 stats) ===============
            h2T = long_pool.tile([P, DC, S], bf16, tag="act2T")
            with ExitStack() as phb:
                pT_pool = phb.enter_context(tc.tile_pool(name="pT", bufs=1))
                sc_ps = phb.enter_context(tc.tile_pool(name="scps", bufs=4,
                                                       space="PSUM"))
                a1_ps = phb.enter_context(tc.tile_pool(name="a1ps", bufs=2,
                                                       space="PSUM"))
                a2_ps = phb.enter_context(tc.tile_pool(name="a2ps", bufs=1,
                                                       space="PSUM"))
                trB_ps = phb.enter_context(tc.tile_pool(name="trBps", bufs=1,
                                                        space="PSUM"))
                rec_pool = phb.enter_context(tc.tile_pool(name="rec", bufs=4))
                statsB = phb.enter_context(tc.tile_pool(name="statsB", bufs=4))

                pT_tiles = {}

                def attn_block(j):
                    pT = pT_tiles.pop(j)
                    for c in range(CPB):
                        scn = j * CPB + c
                        pa1 = a1_ps.tile([P, 256], f32, tag="pa1")
                        pa2 = a2_ps.tile([P, 258], f32, tag="pa2")
                        for m in range(SC // 2):
                            nc.tensor.matmul(pa1[:],
                                             pT[:, 2 * m:2 * m + 2,
                                                c * P:(c + 1) * P],
                                             v_aug[:, 2 * m:2 * m + 2, 0:256],
                                             start=(m == 0),
                                             stop=(m == SC // 2 - 1),
                                             perf_mode=PM2)
                            nc.tensor.matmul(pa2[:],
                                             pT[:, 2 * m:2 * m + 2,
                                                c * P:(c + 1) * P],
                                             v_aug[:, 2 * m:2 * m + 2,
                                                   256:514],
                                             start=(m == 0),
                                             stop=(m == SC // 2 - 1),
                                             perf_mode=PM2)
                        rec = rec_pool.tile([P, 1], f32, tag="rec")
                        nc.vector.reciprocal(rec[:], pa2[:, 256:257])
                        nc.vector.scalar_tensor_tensor(
                            out=xb[:, scn, 0:256], in0=pa1[:], scalar=rec[:],
                            in1=xb[:, scn, 0:256], op0=OP.mult, op1=OP.add)
                        nc.vector.scalar_tensor_tensor(
                            out=xb[:, scn, 256:512], in0=pa2[:, 0:256],
                            scalar=rec[:], in1=xb[:, scn, 256:512],
                            op0=OP.mult, op1=OP.add)
                        # LN2 stats for this finished chunk (batched sqrt later)
                        stats = statsB.tile([P, 6], f32, tag="bn2")
                        nc.vector.bn_stats(stats[:], xb[:, scn, :])
                        nc.vector.bn_aggr(mv_all[:, scn, :], stats[:])

                def ln2_sqrt_group(g):
                    # one Sqrt+Reciprocal per 8 chunks: rstd ready before C
                    sl = slice(8 * g, 8 * (g + 1))
                    nc.scalar.activation(std_all[:, sl], mv_all[:, sl, 1],
                                         AF.Sqrt, bias=eps_t[:], scale=1.0)
                    nc.vector.reciprocal(rstd_all[:, sl], std_all[:, sl])

                for j in range(NB):
                    # scores^T for block j (fp8 DoubleRow), one exp per t-chunk
                    pT = pT_pool.tile([P, SC, SB], f8, tag="pT")
                    pT_tiles[j] = pT
                    for m in range(SC):
                        ps = sc_ps.tile([P, SB], f32, tag="sc")
                        for k in range(DC // 2):
                            nc.tensor.matmul(
                                ps[:],
                                uT[:, 2 * k:2 * k + 2, m * P:(m + 1) * P],
                                hT[:, 2 * k:2 * k + 2, j * SB:(j + 1) * SB],
                                start=(k == 0), stop=(k == DC // 2 - 1),
                                perf_mode=PM2)
                        nc.scalar.activation(pT[:, m, :], ps[:],
                                             AF.Exp, bias=lnc_t[:],
                                             scale=ATTN_SCALE)
                    # single pT buffer: attention follows within the block
                    attn_block(j)
                    if j == 1:
                        ln2_sqrt_group(0)
                        # LN2 apply+transpose for chunks 0..7 inside B:
                        # their hT columns were last read by scores
                        # blocks 0..1 (done), so h2T writes are legal.
                        for i in range(8):
                            h_t = ln_apply(tmpBC, i, mv_all[:, i, 0:1],
                                           rstd_all[:, i:i + 1], 2)
                            transpose_to(trB_ps, h2T, i, h_t)
                ln2_sqrt_group(1)

        # ================= phase C: LN2 apply + MLP =======================
        with ExitStack() as phc:
            wC_pool = phc.enter_context(tc.tile_pool(name="wC", bufs=1))
            gT_pool = phc.enter_context(tc.tile_pool(name="gT", bufs=2))
            tr_psC = phc.enter_context(tc.tile_pool(name="trpsC", bufs=2,
                                                    space="PSUM"))
            f1_ps = phc.enter_context(tc.tile_pool(name="f1ps", bufs=4,
                                                   space="PSUM"))
            y_ps = phc.enter_context(tc.tile_pool(name="yps", bufs=2,
                                                  space="PSUM"))

            # W2 split across the SP and Pool queues for fast arrival
            w2_t = wC_pool.tile([P, FC, D], bf16)
            for g in range(4):
                eng = nc.sync if g % 2 == 0 else nc.gpsimd
                eng.dma_start(w2_t[:, 4 * g:4 * (g + 1), :],
                              w2_r[:, 4 * g:4 * (g + 1), :])

            for i in range(8, SC):
                h_t = ln_apply(tmpBC, i, mv_all[:, i, 0:1],
                               rstd_all[:, i:i + 1], 2)
                transpose_to(tr_psC, h2T, i, h_t)

            def fc2_chunk(jj, gT, c):
                scn = jj * CPBM + c
                ps = y_ps.tile([P, D], f32, tag="y")
                for m in range(FC):
                    nc.tensor.matmul(ps[:], gT[:, m, c * P:(c + 1) * P],
                                     w2_t[:, m, :],
                                     start=(m == 0), stop=(m == FC - 1))
                nc.vector.tensor_tensor(xb[:, scn, :], ps[:],
                                        xb[:, scn, :], op=OP.add)
                nc.vector.tensor_tensor(xb[:, scn, :], xb[:, scn, :],
                                        b2_t[:], op=OP.add)
                eng = nc.sync if scn % 2 == 0 else nc.gpsimd
                eng.dma_start(out_r[:, scn, :], xb[:, scn, :])

            gT_tiles = {}
            for jj in range(NBM):
                gT = gT_pool.tile([P, FC, SBM], bf16, tag="gT")
                gT_tiles[jj] = gT
                for m in range(FC):
                    ps = f1_ps.tile([P, SBM], f32, tag="f1")
                    for k in range(DC):
                        nc.tensor.matmul(ps[:], w1_t[:, k, m * P:(m + 1) * P],
                                         h2T[:, k, jj * SBM:(jj + 1) * SBM],
                                         start=(k == 0), stop=(k == DC - 1))
                    nc.scalar.activation(gT[:, m, :], ps[:], AF.Gelu,
                                         bias=bf_t[:, m:m + 1], scale=1.0)
                    # software pipelining: interleave previous block's fc2
                    if jj > 0 and m % 4 == 3:
                        fc2_chunk(jj - 1, gT_tiles[jj - 1], m // 4)
                if jj > 0:
                    gT_tiles.pop(jj - 1)
            for c in range(CPBM):
                fc2_chunk(NBM - 1, gT_tiles[NBM - 1], c)

    nc.compile()
    return nc


def _fold_weights(inputs):
    """Host-side constant folding (float64): Wu = Wk Wq^T, Wvo = Wv Wo.

    Returns the two packed flat buffers the kernel consumes.
    """
    import ml_dtypes
    f64 = {k: np.asarray(v, dtype=np.float64) for k, v in inputs.items()}
    wu = (f64["Wk"] @ f64["Wq"].T).astype(np.float32)
    wvo = (f64["Wv"] @ f64["Wo"]).astype(np.float32)
    wpack8 = np.clip(
        np.concatenate([wu.ravel(), wvo.ravel()]), -240.0, 240.0
    ).astype(ml_dtypes.float8_e4m3)
    wpack = np.concatenate([
        f64["W1"].ravel(), f64["W2"].ravel(),
    ]).astype(ml_dtypes.bfloat16)
    cpack = np.concatenate([
        f64["b1"].astype(np.float32),
        f64["b2"].astype(np.float32),
        f64["g1"].astype(np.float32),
        f64["be1"].astype(np.float32),
        f64["g2"].astype(np.float32),
        f64["be2"].astype(np.float32),
    ])
    return {"wpack8": wpack8, "wpack": wpack, "cpack": cpack}


def _flags(inputs):
    has1 = not (np.all(np.asarray(inputs["g1"]) == 1.0)
                and np.all(np.asarray(inputs["be1"]) == 0.0))
    has2 = not (np.all(np.asarray(inputs["g2"]) == 1.0)
                and np.all(np.asarray(inputs["be2"]) == 0.0))
    return has1, has2


def _get_runner(flags):
    """Build (once per flag set) a cached jitted SPMD runner over 8 cores."""
    key = ("runner", flags)
    if key in _CACHE:
        return _CACHE[key]

    import jax
    import numpy as _np
    from jax.sharding import Mesh, PartitionSpec, NamedSharding
    from jax.experimental.shard_map import shard_map
    import concourse.mybir as mybir
    from concourse.bass2jax import (_bass_exec_p, install_neuronx_cc_hook,
                                    partition_id_tensor)
    try:
        from concourse.bass2jax import fast_dispatch_compile
    except ImportError:
        fast_dispatch_compile = None

    nc = _build(*flags)
    install_neuronx_cc_hook()

    partition_name = (nc.partition_id_tensor.name
                      if nc.partition_id_tensor else None)
    in_names, out_names, out_avals, zero_outs = [], [], [], []
    in_shapes = {}
    for alloc in nc.m.functions[0].allocations:
        if not isinstance(alloc, mybir.MemoryLocationSet):
            continue
        name = alloc.memorylocations[0].name
        if alloc.kind == "ExternalInput":
            if name != partition_name:
                in_names.append(name)
                in_shapes[name] = (tuple(alloc.tensor_shape),
                                   mybir.dt.np(alloc.dtype))
        elif alloc.kind == "ExternalOutput":
            out_names.append(name)
            shape = tuple(alloc.tensor_shape)
            dtype = mybir.dt.np(alloc.dtype)
            out_avals.append(jax.core.ShapedArray(shape, dtype))
            zero_outs.append(_np.zeros(shape, dtype))
    n_params = len(in_names)
    all_in_names = in_names + out_names
    if partition_name is not None:
        all_in_names = all_in_names + [partition_name]

    def _body(*args):
        operands = list(args)
        if partition_name is not None:
            operands.append(partition_id_tensor())
        outs = _bass_exec_p.bind(
            *operands,
            out_avals=tuple(out_avals),
            in_names=tuple(all_in_names),
            out_names=tuple(out_names),
            lowering_input_output_aliases=(),
            sim_require_finite=True,
            sim_require_nnan=True,
            nc=nc,
        )
        return tuple(outs)

    devices = jax.devices()[:NCORES]
    mesh = Mesh(_np.asarray(devices), ("core",))
    n_all = n_params + len(out_names)

    def _make_jit():
        return jax.jit(
            shard_map(_body, mesh=mesh,
                      in_specs=(PartitionSpec("core"),) * n_all,
                      out_specs=(PartitionSpec("core"),) * len(out_names),
                      check_rep=False),
            keep_unused=True,
        )

    sharding = NamedSharding(mesh, PartitionSpec("core"))

    # bass_exec declares a jax effect, which forces the slow python dispatch
    # path (~1 ms/call host overhead). fast_dispatch_compile suppresses it and
    # AOT-compiles, enabling C++ fast-path dispatch (~0.1 ms/call).
    sharded = None
    if fast_dispatch_compile is not None:
        in_structs = []
        for name in in_names:
            shape, dtype = in_shapes[name]
            in_structs.append(jax.ShapeDtypeStruct(
                (NCORES * shape[0],) + tuple(shape[1:]), dtype,
                sharding=sharding))
        for z in zero_outs:
            in_structs.append(jax.ShapeDtypeStruct(
                (NCORES * z.shape[0],) + tuple(z.shape[1:]), z.dtype,
                sharding=sharding))
        try:
            sharded = fast_dispatch_compile(
                lambda: _make_jit().lower(*in_structs).compile())
        except Exception:
            sharded = None
    if sharded is None:
        sharded = _make_jit()

    # Hot-path call that skips the per-call safety-net shard walk (outputs
    # are always read via block_until_ready, so errors still surface there).
    raw_call = None
    try:
        import jax._src.stages as _jstages
        if isinstance(sharded, _jstages.Compiled):
            raw_call = _jstages.Compiled.__call__.__get__(sharded)
    except Exception:
        raw_call = None
    runner = {
        "sharded": sharded, "sharding": sharding, "in_names": in_names,
        "out_names": out_names, "zero_outs": zero_outs, "jax": jax,
        "np": _np, "raw_call": raw_call,
    }
    _CACHE[key] = runner
    return runner


def _stage(inputs):
    """Shard + fold inputs, return staged device arrays for the runner."""
    flags = _flags(inputs)
    r = _get_runner(flags)
    jax, _np = r["jax"], r["np"]
    x = _np.asarray(inputs["x"], dtype=_np.float32)          # [8, 2048, 512]
    folded = _fold_weights(inputs)
    per_core = {"x": [x[c] for c in range(NCORES)]}
    for k, v in folded.items():
        per_core[k] = [v] * NCORES
    concat = []
    for name in r["in_names"]:
        concat.append(_np.concatenate([per_core[name][c] for c in range(NCORES)],
                                      axis=0))
    for z in r["zero_outs"]:
        concat.append(_np.zeros((NCORES * z.shape[0],) + z.shape[1:], z.dtype))
    return flags, [jax.device_put(a, r["sharding"]) for a in concat]


def _run_staged(flags, staged):
    r = _get_runner(flags)
    call = r.get("raw_call")
    if call is not None:
        return call(*staged)
    return r["sharded"](*staged)


def kernel(**inputs):
    flags, staged = _stage(inputs)
    outs = _run_staged(flags, staged)
    out = np.asarray(outs[0])                                # [8*2048, 512]
    return out.reshape(NCORES, S, D).astype(np.float32)



# revision 10
# speedup vs baseline: 9.2141x; 9.2141x over previous
"""MiniTransformer block on 8 Trainium2 NeuronCores.

Sharding: pure data-parallel over batch (B=8 -> 1 batch element per core,
no collectives). Per core the full transformer block (LN -> single-head
attention -> residual -> LN -> MLP -> residual) runs as one Bass/Tile kernel.

Key design points:
  * All matmuls run in float32r (TF32-like, 1 cycle/row on the PE at free
    dim >= 256 vs 4 cycles/row for fp32; measured fro rel err ~1.5e-4).
  * Activations for matmul consumption are kept transposed ([feature, token])
    so projections chain without transposes; only LN outputs are transposed
    (PE transpose, 4 per 128-row chunk).
  * Attention scores are computed via a host-folded Wu = Wk @ Wq^T:
    scores^T = (h Wu) . h, so only ONE projection (u) is materialized
    instead of q and k.
  * Softmax: scores are computed transposed [t, s]; exp (with the 1/sqrt(D)
    scale fused) happens on the ScalarE during PSUM eviction; no max
    subtraction (LN-bounded scores, fp32 exp range is ample); the
    denominator comes from an extra ones-column appended to v, landing in
    PSUM as a per-partition scalar; normalization + residual add fold into
    a single scalar_tensor_tensor eviction.
  * (p @ v) @ Wo is computed as p @ (v (Wv Wo)) via host-folded Wvo,
    removing a projection and a transpose.
  * The attention side (u/v projections, scores, p@v) runs in fp8e4m3 with
    DoubleRow matmuls (2 contraction rows/cycle, 2x the f32r rate). LN keeps
    activations in fp8 range; exp is biased by ln(1/16) so p~ = exp(s/23)/16
    stays < 240 (fp8 max) -- softmax normalization cancels the constant.
    Attention here is diffuse (scores O(1)), so fp8 score noise averages out.
    The MLP runs with bf16 weights/activations (W1/W2/h2T/gT) but f32 PSUM
    accumulation -- fp8 there would cost ~3e-2 rel err (quantization noise of
    a random linear map does not average down), bf16 only ~5e-4. Measured
    total rel err 3.9e-3 (vs 1.2e-4 all-f32r), budget 2e-2.
  * DMA traffic is spread across the SP/ACT HWDGE queues and the Pool SWDGE
    queue so no engine's sequencer stalls compute.

Host/dispatch side (the measured wall time is dominated by the axon tunnel's
~80 ms round-trip latency plus per-call dispatch cost, not kernel time):
  * Inputs are packed into 3 operands (x, fp8 wpack8, f32 wpack + cpack) so
    the per-call operand marshaling is minimal.
  * The runner is AOT-compiled via fast_dispatch_compile, which suppresses
    the bass_exec jax effect and enables C++ fast-path dispatch (~0.2 ms/call
    vs ~1.2 ms on the effectful python path).
"""

import numpy as np

S, D, F, P = 2048, 512, 2048, 128
SC, DC, FC = S // P, D // P, F // P  # 16, 4, 16
SB = 512                             # attention s-block
NB = S // SB                         # 4
CPB = SB // P                        # s-chunks per attention block = 4
SBM = 512                            # MLP s-block
NBM = S // SBM                       # 4
CPBM = SBM // P                      # s-chunks per MLP block = 4
NCORES = 8
LN_EPS = 1e-5
ATTN_SCALE = float(1.0 / np.sqrt(np.float32(D)))
LOG_EXP_C = float(np.log(1.0 / 16.0))

_CACHE = {}


def _build(has_affine1, has_affine2):
    import concourse.bass as bass
    import concourse.mybir as mybir
    import concourse.tile as tile
    from concourse import bacc
    from concourse.masks import make_identity
    from contextlib import ExitStack

    f32 = mybir.dt.float32
    f32r = mybir.dt.float32r
    f8 = mybir.dt.float8e4
    bf16 = mybir.dt.bfloat16
    i32 = mybir.dt.int32
    PM2 = mybir.MatmulPerfMode.DoubleRow
    AF = mybir.ActivationFunctionType
    OP = mybir.AluOpType

    nc = bacc.Bacc("TRN2", target_bir_lowering=False, debug=False,
                   num_devices=NCORES)

    # Inputs are packed into three flat buffers (x aside) so the per-call
    # dispatch streams 5 operand handles instead of 13: "wpack8" carries the
    # fp8 attention weights, "wpack" the bf16 MLP weights, "cpack" the six
    # small f32 vectors. Views with the original access patterns are
    # hand-built APs at element offsets.
    x_d = nc.dram_tensor("x", [S, D], f32, kind="ExternalInput").ap()
    wpack8_d = nc.dram_tensor("wpack8", [2 * D * D], f8,
                              kind="ExternalInput").ap()
    wpack_d = nc.dram_tensor("wpack", [2 * D * F], bf16,
                             kind="ExternalInput").ap()
    cpack_d = nc.dram_tensor("cpack", [F + 5 * D], f32,
                             kind="ExternalInput").ap()
    out_d = nc.dram_tensor("out", [S, D], f32, kind="ExternalOutput").ap()

    def w8view(base, dims):
        return bass.AP(tensor=wpack8_d.tensor, offset=base,
                       ap=[[s, n] for s, n in dims])

    def wview(base, dims):
        return bass.AP(tensor=wpack_d.tensor, offset=base,
                       ap=[[s, n] for s, n in dims])

    def cview(base, dims):
        return bass.AP(tensor=cpack_d.tensor, offset=base,
                       ap=[[s, n] for s, n in dims])

    x_r = x_d.rearrange("(sc p) d -> p sc d", p=P)      # [128, 16, 512]
    out_r = out_d.rearrange("(sc p) d -> p sc d", p=P)
    # layouts match the originals: wu/wvo [D,D] "(ko ki) n -> ki ko n",
    # w1 [D,F] likewise, w2 [F,D] likewise (ki = partition dim = 128)
    wu_r = w8view(0, [(D, P), (P * D, DC), (1, D)])         # [128, 4, 512]
    wvo_r = w8view(D * D, [(D, P), (P * D, DC), (1, D)])
    w1_r = wview(0, [(F, P), (P * F, DC), (1, F)])          # [128, 4, 2048]
    w2_r = wview(D * F, [(D, P), (P * D, FC), (1, D)])
    bf_r = cview(0, [(1, P), (P, FC)])                      # [128, 16]
    b2_d = cview(F, [(0, P), (1, D)])                       # bcast views
    g1_d = cview(F + D, [(0, P), (1, D)])
    be1_d = cview(F + 2 * D, [(0, P), (1, D)])
    g2_d = cview(F + 3 * D, [(0, P), (1, D)])
    be2_d = cview(F + 4 * D, [(0, P), (1, D)])

    def bcast(ap):  # packed views above are already partition-broadcast
        return ap

    with tile.TileContext(nc) as tc, ExitStack() as top:
        long_pool = top.enter_context(tc.tile_pool(name="long", bufs=1))
        const_pool = top.enter_context(tc.tile_pool(name="consts", bufs=1))
        w1_pool = top.enter_context(tc.tile_pool(name="w1p", bufs=1))
        tmpBC = top.enter_context(tc.tile_pool(name="tmpBC", bufs=2))

        # ---- persistent activations / preloaded weights ------------------
        xb = long_pool.tile([P, SC, D], f32)             # x, becomes x2 in place
        hT = long_pool.tile([P, DC, S], f8, tag="actT")  # LN1 output, fp8
        w1_t = w1_pool.tile([P, DC, F], bf16)

        # x chunk DMAs first on both HWDGE queues so nothing gates them.
        for i in range(SC):
            (nc.sync if i % 2 == 0 else nc.scalar).dma_start(
                xb[:, i, :], x_r[:, i, :])

        # ---- constants / small tiles -------------------------------------
        ident = const_pool.tile([P, P], f32r)
        zwarm = const_pool.tile([P, P], f32r)
        nc.vector.memset(zwarm[:].bitcast(f32), 0.0)
        # PE warmup: dependency-free matmuls at t=0 release the HAM throttle
        # (~3.4us of sustained PE activity) before real work arrives.
        with tc.tile_pool(name="warmps", bufs=1, space="PSUM") as wps:
            wm = wps.tile([P, 512], f32)
            for _ in range(16):
                nc.tensor.matmul(wm[:, 0:P], zwarm[:], zwarm[:],
                                 start=True, stop=True)
        with tc.tile_pool(name="identf", bufs=1) as idp:
            ident_f = idp.tile([P, P], f32)
            make_identity(nc, ident_f[:])
            nc.vector.tensor_copy(ident[:], ident_f[:])
        # exp bias ln(1/16): p~ = exp(s/sqrt(D))/16 stays < 240 (fp8e4 max);
        # softmax normalization cancels the constant exactly.
        lnc_t = const_pool.tile([P, 1], f32)
        nc.vector.memset(lnc_t[:], LOG_EXP_C)
        one_f = const_pool.tile([P, 1], f32)
        nc.vector.memset(one_f[:], 1.0)
        # fp8 ones pair for the softmax-denominator matmul row (the 16-wide
        # allocation keeps the DoubleRow k-row step 16B-aligned)
        ones8 = const_pool.tile([P, 2, 16], f8)
        nc.vector.memset(ones8[:], 1.0)

        bf_t = const_pool.tile([P, FC], f32)
        nc.scalar.dma_start(bf_t[:], bf_r)
        b2_t = const_pool.tile([P, D], f32)
        nc.scalar.dma_start(b2_t[:], bcast(b2_d))
        aff = {}
        if has_affine1:
            aff[1] = (const_pool.tile([P, D], f32, tag="g1b"),
                      const_pool.tile([P, D], f32, tag="be1b"))
            nc.scalar.dma_start(aff[1][0][:], bcast(g1_d))
            nc.scalar.dma_start(aff[1][1][:], bcast(be1_d))
        if has_affine2:
            aff[2] = (const_pool.tile([P, D], f32, tag="g2b"),
                      const_pool.tile([P, D], f32, tag="be2b"))
            nc.scalar.dma_start(aff[2][0][:], bcast(g2_d))
            nc.scalar.dma_start(aff[2][1][:], bcast(be2_d))

        # LN2 batched stats (written in phase B, consumed in phase C) and
        # softmax reciprocal denominators
        mv_all = const_pool.tile([P, SC, 2], f32)
        rstd_all = const_pool.tile([P, SC], f32)
        rec_all = const_pool.tile([P, SC], f32)

        def rsqrt_cols(qpool, dst_sl, var_sl, n):
            """dst = 1/sqrt(var + eps) entirely on the DVE.

            Quake-style initial guess computed in float arithmetic on the
            integer VALUE of the fp32 bit pattern (int<->float converts are
            exact to ~2^-24 here, dwarfed by the guess's own ~3.4% error),
            then three Newton iterations (final rel err fp32-limited).
            """
            ve = qpool.tile([P, n], f32, tag=f"q_ve{n}")
            nc.vector.tensor_scalar_add(ve[:], var_sl, LN_EPS)
            fb = qpool.tile([P, n], f32, tag=f"q_fb{n}")
            nc.vector.tensor_copy(fb[:], ve[:].bitcast(i32))  # int->float
            nc.vector.tensor_scalar(out=fb[:], in0=fb[:], scalar1=-0.5,
                                    scalar2=float(0x5F3759DF),
                                    op0=OP.mult, op1=OP.add)
            y = qpool.tile([P, n], f32, tag=f"q_y{n}")
            nc.vector.tensor_copy(y[:].bitcast(i32), fb[:])   # float->int
            t = qpool.tile([P, n], f32, tag=f"q_t{n}")
            for it in range(3):
                nc.vector.tensor_tensor(t[:], y[:], y[:], op=OP.mult)
                nc.vector.tensor_tensor(t[:], t[:], ve[:], op=OP.mult)
                nc.vector.tensor_scalar(out=t[:], in0=t[:], scalar1=-0.5,
                                        scalar2=1.5, op0=OP.mult, op1=OP.add)
                out = dst_sl if it == 2 else y[:]
                nc.vector.tensor_tensor(out, y[:], t[:], op=OP.mult)

        def ln_apply(tmp_pool, i, mean, rstd, which):
            """(xb[:,i,:] - mean) * rstd [*gamma + beta] -> row-major tile."""
            h_t = tmp_pool.tile([P, D], f32r, tag="h_rm")
            nc.vector.tensor_scalar(out=h_t[:], in0=xb[:, i, :],
                                    scalar1=mean, scalar2=rstd,
                                    op0=OP.subtract, op1=OP.mult)
            if which in aff:
                g_b, be_b = aff[which]
                nc.vector.tensor_tensor(h_t[:], h_t[:], g_b[:], op=OP.mult)
                nc.vector.tensor_tensor(h_t[:], h_t[:], be_b[:], op=OP.add)
            return h_t

        def transpose_to(tr_psum, dest_T, i, h_t, evict):
            for dj in range(DC):
                ps = tr_psum.tile([P, P], f32r, tag="tr")
                nc.tensor.transpose(ps[:], h_t[:, dj * P:(dj + 1) * P], ident[:])
                if evict == "scalar":
                    nc.scalar.copy(dest_T[:, dj, i * P:(i + 1) * P], ps[:])
                else:
                    nc.vector.tensor_copy(dest_T[:, dj, i * P:(i + 1) * P],
                                          ps[:])

        # ================= phase A: LN1 + u/v projections =================
        with ExitStack() as ph:
            ph_qk = ph.enter_context(tc.tile_pool(name="uv", bufs=1))
            uT = ph_qk.tile([P, DC, S], f8, tag="uT")
            v_t = ph_qk.tile([P, SC, D], f8, tag="vt")

            with ExitStack() as pha:
                wA_pool = pha.enter_context(tc.tile_pool(name="wA", bufs=1))
                tmpA = pha.enter_context(tc.tile_pool(name="tmpA", bufs=3))
                statsA = pha.enter_context(tc.tile_pool(name="statsA", bufs=4))
                qpoolA = pha.enter_context(tc.tile_pool(name="qpA", bufs=2))
                tr_psA = pha.enter_context(tc.tile_pool(name="trpsA", bufs=2,
                                                        space="PSUM"))
                mm_psA = pha.enter_context(tc.tile_pool(name="mmpsA", bufs=4,
                                                        space="PSUM"))

                wu_t = wA_pool.tile([P, DC, D], f8)
                nc.gpsimd.dma_start(wu_t[:], wu_r)
                wvo_t = wA_pool.tile([P, DC, D], f8)
                nc.gpsimd.dma_start(wvo_t[:], wvo_r)
                # W1 preload early on the idle-after-that Pool queue
                nc.gpsimd.dma_start(w1_t[:], w1_r)

                mv1 = statsA.tile([P, SC, 2], f32, tag="mv1")
                rstd1 = statsA.tile([P, SC], f32, tag="rstd1")
                for i in range(SC):
                    stats = statsA.tile([P, 6], f32, tag="bn_stats")
                    nc.vector.bn_stats(stats[:], xb[:, i, :])
                    nc.vector.bn_aggr(mv1[:, i, :], stats[:])
                    if i % 4 != 3:
                        continue
                    g = i // 4
                    sl = slice(4 * g, 4 * g + 4)
                    rsqrt_cols(qpoolA, rstd1[:, sl], mv1[:, sl, 1], 4)
                    for ii in range(4 * g, 4 * g + 4):
                        h_t = ln_apply(tmpA, ii, mv1[:, ii, 0:1],
                                       rstd1[:, ii:ii + 1], 1)
                        transpose_to(tr_psA, hT, ii, h_t, "scalar")
                        # v' row-major for chunk ii: [t, dout] = h @ Wvo
                        ps = mm_psA.tile([P, 512], f32, tag="proj")
                        for k in range(DC // 2):
                            nc.tensor.matmul(ps[:],
                                             hT[:, 2 * k:2 * k + 2,
                                                ii * P:(ii + 1) * P],
                                             wvo_t[:, 2 * k:2 * k + 2, :],
                                             start=(k == 0),
                                             stop=(k == DC // 2 - 1),
                                             perf_mode=PM2)
                        nc.vector.tensor_copy(v_t[:, ii, :], ps[:])
                    # uT s-tile for this 4-chunk group
                    for m in range(DC):
                        ps = mm_psA.tile([P, 512], f32, tag="proj")
                        for k in range(DC // 2):
                            nc.tensor.matmul(
                                ps[:],
                                wu_t[:, 2 * k:2 * k + 2, m * P:(m + 1) * P],
                                hT[:, 2 * k:2 * k + 2,
                                   g * 512:(g + 1) * 512],
                                start=(k == 0), stop=(k == DC // 2 - 1),
                                perf_mode=PM2)
                        nc.scalar.copy(uT[:, m, g * 512:(g + 1) * 512], ps[:])

            # ============= phase B: attention (+ LN2 stats) ===============
            h2T = long_pool.tile([P, DC, S], bf16, tag="act2T")
            # W2 streamed during B so phase C starts immediately
            w2_t = long_pool.tile([P, FC, D], bf16, tag="w2")
            for gq in range(4):
                eng = nc.sync if gq % 2 == 0 else nc.gpsimd
                eng.dma_start(w2_t[:, 4 * gq:4 * (gq + 1), :],
                              w2_r[:, 4 * gq:4 * (gq + 1), :])
            with ExitStack() as phb:
                pT_pool = phb.enter_context(tc.tile_pool(name="pT", bufs=1))
                sc_ps = phb.enter_context(tc.tile_pool(name="scps", bufs=3,
                                                       space="PSUM"))
                pa_ps = phb.enter_context(tc.tile_pool(name="paps", bufs=2,
                                                       space="PSUM"))
                den_ps = phb.enter_context(tc.tile_pool(name="denps", bufs=1,
                                                        space="PSUM"))
                sm_ps = phb.enter_context(tc.tile_pool(name="smps", bufs=1,
                                                       space="PSUM"))
                den_sbp = phb.enter_context(tc.tile_pool(name="densb", bufs=2))
                statsB = phb.enter_context(tc.tile_pool(name="statsB", bufs=4))
                qpoolB = phb.enter_context(tc.tile_pool(name="qpB", bufs=2))

                for j in range(NB):
                    # scores^T for block j (fp8 DoubleRow), one exp per t-chunk
                    pT = pT_pool.tile([P, SC, SB], f8, tag="pT")
                    den = den_ps.tile([1, SB], f32, tag="den")
                    for m in range(SC):
                        ps = sc_ps.tile([P, SB], f32, tag="sc")
                        for k in range(DC // 2):
                            nc.tensor.matmul(
                                ps[:],
                                uT[:, 2 * k:2 * k + 2, m * P:(m + 1) * P],
                                hT[:, 2 * k:2 * k + 2, j * SB:(j + 1) * SB],
                                start=(k == 0), stop=(k == DC // 2 - 1),
                                perf_mode=PM2)
                        nc.scalar.activation(pT[:, m, :], ps[:],
                                             AF.Exp, bias=lnc_t[:],
                                             scale=ATTN_SCALE)
                        if m % 2 == 1:
                            # denominator row: ones.T @ pT accumulated over t
                            r = m // 2
                            nc.tensor.matmul(
                                den[:], ones8[:, :, 0:1],
                                pT[:, m - 1:m + 1, :],
                                start=(r == 0), stop=(r == SC // 2 - 1),
                                perf_mode=PM2)
                    # den row -> per-chunk reciprocal columns (PE transpose)
                    den_sb = den_sbp.tile([1, SB], f32, tag="densb")
                    nc.vector.tensor_copy(den_sb[:], den[:])
                    for c in range(CPB):
                        trp = sm_ps.tile([P, 1], f32, tag="dtr")
                        nc.tensor.transpose(trp[:],
                                            den_sb[0:1, c * P:(c + 1) * P],
                                            one_f[0:1, 0:1])
                        nc.vector.reciprocal(
                            rec_all[:, j * CPB + c:j * CPB + c + 1], trp[:])
                    # p @ v' with a single N=512 matmul per contraction pair
                    for c in range(CPB):
                        scn = j * CPB + c
                        pa = pa_ps.tile([P, D], f32, tag="pa")
                        for m in range(SC // 2):
                            nc.tensor.matmul(pa[:],
                                             pT[:, 2 * m:2 * m + 2,
                                                c * P:(c + 1) * P],
                                             v_t[:, 2 * m:2 * m + 2, :],
                                             start=(m == 0),
                                             stop=(m == SC // 2 - 1),
                                             perf_mode=PM2)
                        nc.vector.scalar_tensor_tensor(
                            out=xb[:, scn, :], in0=pa[:],
                            scalar=rec_all[:, scn:scn + 1],
                            in1=xb[:, scn, :], op0=OP.mult, op1=OP.add)
                        # LN2 stats for this finished chunk
                        stats = statsB.tile([P, 6], f32, tag="bn2")
                        nc.vector.bn_stats(stats[:], xb[:, scn, :])
                        nc.vector.bn_aggr(mv_all[:, scn, :], stats[:])
                    if j >= 1:
                        # LN2 apply+transpose for block j-1's chunks, spread
                        # across the attention blocks (stats done at j-1)
                        lo = 4 * (j - 1)
                        rsqrt_cols(qpoolB, rstd_all[:, lo:lo + 4],
                                   mv_all[:, lo:lo + 4, 1], 4)
                        for i in range(lo, lo + 4):
                            h_t = ln_apply(tmpBC, i, mv_all[:, i, 0:1],
                                           rstd_all[:, i:i + 1], 2)
                            transpose_to(sm_ps, h2T, i, h_t, "vector")

        # ================= phase C: LN2 apply + MLP =======================
        with ExitStack() as phc:
            gT_pool = phc.enter_context(tc.tile_pool(name="gT", bufs=2))
            qpoolC = phc.enter_context(tc.tile_pool(name="qpC", bufs=1))
            tr_psC = phc.enter_context(tc.tile_pool(name="trpsC", bufs=2,
                                                    space="PSUM"))
            f1_ps = phc.enter_context(tc.tile_pool(name="f1ps", bufs=4,
                                                   space="PSUM"))
            y_ps = phc.enter_context(tc.tile_pool(name="yps", bufs=2,
                                                  space="PSUM"))

            rsqrt_cols(qpoolC, rstd_all[:, 12:SC], mv_all[:, 12:SC, 1], 4)
            for i in range(12, SC):
                h_t = ln_apply(tmpBC, i, mv_all[:, i, 0:1],
                               rstd_all[:, i:i + 1], 2)
                transpose_to(tr_psC, h2T, i, h_t, "vector")
            # batched b2 pre-add into x2 (after the LN2 applies read xb)
            nc.vector.tensor_tensor(
                xb[:, 0:12, :], xb[:, 0:12, :],
                b2_t[:, None, :].to_broadcast((P, 12, D)), op=OP.add)
            nc.vector.tensor_tensor(
                xb[:, 12:SC, :], xb[:, 12:SC, :],
                b2_t[:, None, :].to_broadcast((P, 4, D)), op=OP.add)

            def fc2_chunk(jj, gT, c):
                scn = jj * CPBM + c
                ps = y_ps.tile([P, D], f32, tag="y")
                for m in range(FC):
                    nc.tensor.matmul(ps[:], gT[:, m, c * P:(c + 1) * P],
                                     w2_t[:, m, :],
                                     start=(m == 0), stop=(m == FC - 1))
                nc.vector.tensor_tensor(xb[:, scn, :], ps[:],
                                        xb[:, scn, :], op=OP.add)
                eng = nc.sync if scn % 2 == 0 else nc.gpsimd
                eng.dma_start(out_r[:, scn, :], xb[:, scn, :])

            gT_tiles = {}
            for jj in range(NBM):
                gT = gT_pool.tile([P, FC, SBM], bf16, tag="gT")
                gT_tiles[jj] = gT
                for m in range(FC):
                    ps = f1_ps.tile([P, SBM], f32, tag="f1")
                    for k in range(DC):
                        nc.tensor.matmul(ps[:], w1_t[:, k, m * P:(m + 1) * P],
                                         h2T[:, k, jj * SBM:(jj + 1) * SBM],
                                         start=(k == 0), stop=(k == DC - 1))
                    nc.scalar.activation(gT[:, m, :], ps[:], AF.Gelu,
                                         bias=bf_t[:, m:m + 1], scale=1.0)
                    # software pipelining: interleave previous block's fc2
                    if jj > 0 and m % 4 == 3:
                        fc2_chunk(jj - 1, gT_tiles[jj - 1], m // 4)
                if jj > 0:
                    gT_tiles.pop(jj - 1)
            for c in range(CPBM):
                fc2_chunk(NBM - 1, gT_tiles[NBM - 1], c)

    nc.compile()
    return nc


def _fold_weights(inputs):
    """Host-side constant folding (float64): Wu = Wk Wq^T, Wvo = Wv Wo.

    Returns the two packed flat buffers the kernel consumes.
    """
    import ml_dtypes
    f64 = {k: np.asarray(v, dtype=np.float64) for k, v in inputs.items()}
    wu = (f64["Wk"] @ f64["Wq"].T).astype(np.float32)
    wvo = (f64["Wv"] @ f64["Wo"]).astype(np.float32)
    wpack8 = np.clip(
        np.concatenate([wu.ravel(), wvo.ravel()]), -240.0, 240.0
    ).astype(ml_dtypes.float8_e4m3)
    wpack = np.concatenate([
        f64["W1"].ravel(), f64["W2"].ravel(),
    ]).astype(ml_dtypes.bfloat16)
    cpack = np.concatenate([
        f64["b1"].astype(np.float32),
        f64["b2"].astype(np.float32),
        f64["g1"].astype(np.float32),
        f64["be1"].astype(np.float32),
        f64["g2"].astype(np.float32),
        f64["be2"].astype(np.float32),
    ])
    return {"wpack8": wpack8, "wpack": wpack, "cpack": cpack}


def _flags(inputs):
    has1 = not (np.all(np.asarray(inputs["g1"]) == 1.0)
                and np.all(np.asarray(inputs["be1"]) == 0.0))
    has2 = not (np.all(np.asarray(inputs["g2"]) == 1.0)
                and np.all(np.asarray(inputs["be2"]) == 0.0))
    return has1, has2


def _get_runner(flags):
    """Build (once per flag set) a cached jitted SPMD runner over 8 cores."""
    key = ("runner", flags)
    if key in _CACHE:
        return _CACHE[key]

    import jax
    import numpy as _np
    from jax.sharding import Mesh, PartitionSpec, NamedSharding
    from jax.experimental.shard_map import shard_map
    import concourse.mybir as mybir
    from concourse.bass2jax import (_bass_exec_p, install_neuronx_cc_hook,
                                    partition_id_tensor)
    try:
        from concourse.bass2jax import fast_dispatch_compile
    except ImportError:
        fast_dispatch_compile = None

    nc = _build(*flags)
    install_neuronx_cc_hook()

    partition_name = (nc.partition_id_tensor.name
                      if nc.partition_id_tensor else None)
    in_names, out_names, out_avals, zero_outs = [], [], [], []
    in_shapes = {}
    for alloc in nc.m.functions[0].allocations:
        if not isinstance(alloc, mybir.MemoryLocationSet):
            continue
        name = alloc.memorylocations[0].name
        if alloc.kind == "ExternalInput":
            if name != partition_name:
                in_names.append(name)
                in_shapes[name] = (tuple(alloc.tensor_shape),
                                   mybir.dt.np(alloc.dtype))
        elif alloc.kind == "ExternalOutput":
            out_names.append(name)
            shape = tuple(alloc.tensor_shape)
            dtype = mybir.dt.np(alloc.dtype)
            out_avals.append(jax.core.ShapedArray(shape, dtype))
            zero_outs.append(_np.zeros(shape, dtype))
    n_params = len(in_names)
    all_in_names = in_names + out_names
    if partition_name is not None:
        all_in_names = all_in_names + [partition_name]

    def _body(*args):
        operands = list(args)
        if partition_name is not None:
            operands.append(partition_id_tensor())
        outs = _bass_exec_p.bind(
            *operands,
            out_avals=tuple(out_avals),
            in_names=tuple(all_in_names),
            out_names=tuple(out_names),
            lowering_input_output_aliases=(),
            sim_require_finite=True,
            sim_require_nnan=True,
            nc=nc,
        )
        return tuple(outs)

    devices = jax.devices()[:NCORES]
    mesh = Mesh(_np.asarray(devices), ("core",))
    n_all = n_params + len(out_names)

    def _make_jit():
        return jax.jit(
            shard_map(_body, mesh=mesh,
                      in_specs=(PartitionSpec("core"),) * n_all,
                      out_specs=(PartitionSpec("core"),) * len(out_names),
                      check_rep=False),
            keep_unused=True,
        )

    sharding = NamedSharding(mesh, PartitionSpec("core"))

    # bass_exec declares a jax effect, which forces the slow python dispatch
    # path (~1 ms/call host overhead). fast_dispatch_compile suppresses it and
    # AOT-compiles, enabling C++ fast-path dispatch (~0.1 ms/call).
    sharded = None
    if fast_dispatch_compile is not None:
        in_structs = []
        for name in in_names:
            shape, dtype = in_shapes[name]
            in_structs.append(jax.ShapeDtypeStruct(
                (NCORES * shape[0],) + tuple(shape[1:]), dtype,
                sharding=sharding))
        for z in zero_outs:
            in_structs.append(jax.ShapeDtypeStruct(
                (NCORES * z.shape[0],) + tuple(z.shape[1:]), z.dtype,
                sharding=sharding))
        try:
            sharded = fast_dispatch_compile(
                lambda: _make_jit().lower(*in_structs).compile())
        except Exception:
            sharded = None
    if sharded is None:
        sharded = _make_jit()

    # Hot-path call that skips the per-call safety-net shard walk (outputs
    # are always read via block_until_ready, so errors still surface there).
    raw_call = None
    try:
        import jax._src.stages as _jstages
        if isinstance(sharded, _jstages.Compiled):
            raw_call = _jstages.Compiled.__call__.__get__(sharded)
    except Exception:
        raw_call = None
    runner = {
        "sharded": sharded, "sharding": sharding, "in_names": in_names,
        "out_names": out_names, "zero_outs": zero_outs, "jax": jax,
        "np": _np, "raw_call": raw_call,
    }
    _CACHE[key] = runner
    return runner


def _stage(inputs):
    """Shard + fold inputs, return staged device arrays for the runner."""
    flags = _flags(inputs)
    r = _get_runner(flags)
    jax, _np = r["jax"], r["np"]
    x = _np.asarray(inputs["x"], dtype=_np.float32)          # [8, 2048, 512]
    folded = _fold_weights(inputs)
    per_core = {"x": [x[c] for c in range(NCORES)]}
    for k, v in folded.items():
        per_core[k] = [v] * NCORES
    concat = []
    for name in r["in_names"]:
        concat.append(_np.concatenate([per_core[name][c] for c in range(NCORES)],
                                      axis=0))
    for z in r["zero_outs"]:
        concat.append(_np.zeros((NCORES * z.shape[0],) + z.shape[1:], z.dtype))
    return flags, [jax.device_put(a, r["sharding"]) for a in concat]


def _run_staged(flags, staged):
    r = _get_runner(flags)
    call = r.get("raw_call")
    if call is not None:
        return call(*staged)
    return r["sharded"](*staged)


def kernel(**inputs):
    flags, staged = _stage(inputs)
    outs = _run_staged(flags, staged)
    out = np.asarray(outs[0])                                # [8*2048, 512]
    return out.reshape(NCORES, S, D).astype(np.float32)

